# revision 11
# baseline (speedup 1.0000x reference)
"""Multi-head causal attention (B=4, T=2048, C=512, H=8, HS=64) on 8 TRN2 cores.

Sharding: 2D (batch x head-group). Core (b, g) = core 2*b+g handles batch b and
heads 4g..4g+3, producing a partial output y_bg = sum_h softmax(q_h k_h^T) v_h
@ Wp[64h:64h+64]. Host sums the two head-group partials per batch.

Per-core kernel layout (all matmuls fp32r, contraction dim on partitions):
  - x^T  [c, t] supplied pre-transposed from host (c-chunked) so projections
    contract over c without on-chip transposes.
  - q^T/k^T [d, t] per head-pair stacked on partitions (head a rows 0:64,
    head b rows 64:128) -> scores S^T[tk, tq] = k-block @ q^T via
    lhsT=k^T-block, rhs=q^T (both operands at the same base partition).
  - softmax without max-subtraction (scores ~ N(0,1): exp is safe);
    exp only over the causally-valid column range; per-chunk triangle zeroed
    with gpsimd affine_select; denominators come free as row 64 of the
    attention output via a ones-column appended to v (lhsT M=65).
  - out^T normalized by 1/sums (DVE reciprocal_approx_fast + stride-0-DMA
    partition broadcast), then y[tq, c] = (outn-pair).T @ Wp-pair with two
    heads stacked on the contraction dim (K=128).
"""

import numpy as np

import concourse.bass as bass
import concourse.mybir as mybir
import concourse.tile as tile
from concourse import bacc
from concourse import bass_utils

F32 = mybir.dt.float32
F32R = mybir.dt.float32r

B, T, C, H, HS = 4, 2048, 512, 8, 64
NCC = 4          # 128-row chunks of C
NJ = 16          # 128-row tk chunks of T
NS = 4           # 512-col tq slices of T
SCALE = HS ** -0.5


def r(ap):
    return ap


def build_mha_kernel(tc):
    nc = tc.nc
    xT_d = nc.dram_tensor("xT", [128, NCC, T], F32R, kind="ExternalInput").ap()
    wqk_d = nc.dram_tensor("wqk", [128, 2, 2, NCC, 128], F32R, kind="ExternalInput").ap()
    wv_d = nc.dram_tensor("wv", [128, NCC, 4, 65], F32R, kind="ExternalInput").ap()
    wp_d = nc.dram_tensor("wp", [128, 2, C], F32R, kind="ExternalInput").ap()
    ones_d = nc.dram_tensor("ones", [128, NJ, 4, 1], F32R, kind="ExternalInput").ap()
    y_d = nc.dram_tensor("y", [T, C], F32, kind="ExternalOutput").ap()

    with (
        tc.tile_pool(name="big", bufs=1) as bigp,
        tc.tile_pool(name="pslab", bufs=2) as pslabp,
        tc.tile_pool(name="outup", bufs=2) as outup,
        tc.tile_pool(name="recipp", bufs=2) as recipp,
        tc.tile_pool(name="ysb", bufs=3) as ysbp,
    ):
        xs = bigp.tile([128, NCC, T], F32R)
        nc.sync.dma_start(out=xs, in_=xT_d)
        wqk_s = bigp.tile([128, 2, 2, NCC, 128], F32R)
        nc.sync.dma_start(out=wqk_s, in_=wqk_d)
        wv_s = bigp.tile([128, NCC, 4, 65], F32R)
        nc.sync.dma_start(out=wv_s, in_=wv_d)
        wp_s = bigp.tile([128, 2, C], F32R)
        nc.sync.dma_start(out=wp_s, in_=wp_d)

        qT = bigp.tile([128, 2, T], F32R)      # [d-pair, pr, t]
        kT = bigp.tile([128, 2, T], F32R)
        v4e = bigp.tile([128, NJ, 4, 65], F32R)  # [t-in-chunk, j, head, d|1]
        outn = bigp.tile([128, 2, T], F32R)    # normalized out^T, head pairs stacked

        # ---- q/k projections: two heads packed in M ----
        with tc.tile_pool(name="pj", bufs=4, space="PSUM") as pjp:
            for pr in range(2):
                for qk in range(2):
                    tiles = []
                    for ts in range(NS):
                        qk_ps = pjp.tile([128, 512], F32, tag="qk", name=f"qk_{pr}_{qk}_{ts}")
                        tiles.append(qk_ps)
                    for cc in range(NCC):
                        for ts in range(NS):
                            nc.tensor.matmul(
                                tiles[ts],
                                r(wqk_s[:, pr, qk, cc, :]),
                                r(xs[:, cc, 512 * ts:512 * (ts + 1)]),
                                start=(cc == 0), stop=(cc == NCC - 1),
                            )
                    dst = qT if qk == 0 else kT
                    for ts in range(NS):
                        nc.vector.tensor_copy(
                            out=dst[:, pr, 512 * ts:512 * (ts + 1)], in_=tiles[ts])

            # ---- v projection: 4 heads packed in N (260 cols) ----
            for j in range(NJ):
                v_ps = pjp.tile([128, 4, 65], F32, tag="v", name=f"v_{j}")
                for cc in range(NCC):
                    nc.tensor.matmul(
                        v_ps,
                        r(xs[:, cc, 128 * j:128 * (j + 1)]),
                        r(wv_s[:, cc, :, :]),
                        start=(cc == 0), stop=(cc == NCC - 1),
                    )
                nc.vector.tensor_copy(out=v4e[:, j, :, :], in_=v_ps)
            # ones column for the av-sums row (after the copies that overwrite it)
            nc.sync.dma_start(out=v4e[:, :, :, 64:65], in_=ones_d)

        # ---- attention, head by head ----
        for hh in range(4):
            pr, lo = hh // 2, (hh % 2) * 64
            q_h = qT[lo:lo + 64, pr, :]
            k_h = kT[lo:lo + 64, pr, :]

            with (
                tc.tile_pool(name=f"ot{hh}", bufs=1, space="PSUM") as otp,
                tc.tile_pool(name=f"sp{hh}", bufs=2, space="PSUM") as spp,
            ):
                ot = otp.tile([65, T], F32, name=f"ot_{hh}")

                p_tiles = [None] * NJ

                def emit_scores(j):
                    smin = j // 4
                    stiles = {}
                    for p in range((smin) // 2, 2):
                        stiles[p] = spp.tile([128, 1024], F32, tag="s",
                                             name=f"s_{hh}_{j}_{p}")
                    for s in range(smin, NS):
                        nc.tensor.matmul(
                            stiles[s // 2][:, (s % 2) * 512:(s % 2) * 512 + 512],
                            r(k_h[:, 128 * j:128 * (j + 1)]),
                            r(q_h[:, 512 * s:512 * (s + 1)]),
                            start=True, stop=True,
                        )
                    pt = pslabp.tile([128, T], F32R, tag="p", name=f"p_{hh}_{j}")
                    p_tiles[j] = pt
                    m = j % 4
                    for p in sorted(stiles):
                        st = max(0, 128 * j - 1024 * p)
                        nc.scalar.activation(
                            out=pt[:, 1024 * p + st:1024 * (p + 1)],
                            in_=stiles[p][:, st:1024],
                            func=mybir.ActivationFunctionType.Exp,
                            scale=SCALE,
                        )
                    if m > 0:
                        # zero cols left of the diagonal block (predicate
                        # always false -> fill; dummy initialized input)
                        nc.gpsimd.affine_select(
                            out=pt[:, 512 * smin:512 * smin + 128 * m],
                            in_=xs[:, 0, 0:128 * m],
                            compare_op=mybir.AluOpType.is_ge,
                            fill=0.0, base=-1, channel_multiplier=0,
                            pattern=[[0, 128 * m]],
                        )
                    # zero the upper triangle of the diagonal 128x128 block:
                    # keep (i, jj) iff jj >= i
                    nc.gpsimd.affine_select(
                        out=pt[:, 128 * j:128 * (j + 1)],
                        in_=pt[:, 128 * j:128 * (j + 1)],
                        compare_op=mybir.AluOpType.is_ge,
                        fill=0.0, base=0, channel_multiplier=-1,
                        pattern=[[1, 128]],
                    )

                def emit_av(j):
                    pt = p_tiles[j]
                    for s in range(j // 4, NS):
                        nc.tensor.matmul(
                            ot[:, 512 * s:512 * (s + 1)],
                            r(v4e[:, j, hh, :]),
                            r(pt[:, 512 * s:512 * (s + 1)]),
                            start=(j == 0), stop=(j == 4 * s + 3),
                        )

                emit_scores(0)
                for j in range(1, NJ):
                    emit_scores(j)
                    emit_av(j - 1)
                emit_av(NJ - 1)

                # stash raw out^T (rows 0:64) + sums (row 64) to SBUF
                outu = outup.tile([65, T], F32, tag="outu", name=f"outu_{hh}")
                nc.vector.tensor_copy(out=outu, in_=ot)

            # sums row -> partition 0 (recip/broadcast ignore AP partition
            # offsets on HW), reciprocal, broadcast to 64 partitions
            sums0 = recipp.tile([1, T], F32, tag="sums0", name=f"sums0_{hh}")
            nc.sync.dma_start(out=sums0, in_=outu[64:65, :])
            nc.vector.reciprocal_approx_fast(out=sums0, in_=sums0)
            recipb = recipp.tile([64, T], F32, tag="recipb", name=f"recipb_{hh}")
            nc.gpsimd.partition_broadcast(recipb, sums0)
            if hh % 2 == 0:
                nc.vector.tensor_mul(outn[0:64, pr, :], outu[0:64, :], recipb)
            else:
                nc.vector.tensor_mul(recipb, outu[0:64, :], recipb)
                nc.gpsimd.dma_start(out=outn[64:128, pr, :], in_=recipb)

        # ---- output projection: y[tq, c] = sum_pr outn_pr.T @ wp_pr ----
        with tc.tile_pool(name="yp", bufs=3, space="PSUM") as ypp:
            for ch in range(NJ):
                y_ps = ypp.tile([128, C], F32, tag="y", name=f"y_{ch}")
                for pr in range(2):
                    nc.tensor.matmul(
                        y_ps,
                        r(outn[:, pr, 128 * ch:128 * (ch + 1)]),
                        r(wp_s[:, pr, :]),
                        start=(pr == 0), stop=(pr == 1),
                    )
                y_sb = ysbp.tile([128, C], F32, tag="ysb", name=f"ysb_{ch}")
                nc.vector.tensor_copy(out=y_sb, in_=y_ps)
                nc.sync.dma_start(out=y_d[128 * ch:128 * (ch + 1), :], in_=y_sb)


_PROGRAM = None


def get_program():
    global _PROGRAM
    if _PROGRAM is None:
        nc = bacc.Bacc("TRN2", target_bir_lowering=False, debug=False)
        with tile.TileContext(nc) as tc:
            build_mha_kernel(tc)
        nc.compile()
        _PROGRAM = nc
    return _PROGRAM


def prep_in_maps(x, Wq, Wk, Wv, Wp):
    x = np.asarray(x, np.float32)
    Wq = np.asarray(Wq, np.float32)
    Wk = np.asarray(Wk, np.float32)
    Wv = np.asarray(Wv, np.float32)
    Wp = np.asarray(Wp, np.float32)

    in_maps = []
    for core in range(8):
        b, g = core // 2, core % 2
        hs = [4 * g + i for i in range(4)]

        xT = np.ascontiguousarray(
            x[b].T.reshape(NCC, 128, T).transpose(1, 0, 2))  # [128, cc, t]

        wqk = np.empty((128, 2, 2, NCC, 128), np.float32)
        for pr in range(2):
            ha, hb = hs[2 * pr], hs[2 * pr + 1]
            for cc in range(NCC):
                sl = slice(cc * 128, cc * 128 + 128)
                wqk[:, pr, 0, cc, 0:64] = Wq[ha][sl]
                wqk[:, pr, 0, cc, 64:128] = Wq[hb][sl]
                wqk[:, pr, 1, cc, 0:64] = Wk[ha][sl]
                wqk[:, pr, 1, cc, 64:128] = Wk[hb][sl]

        wv = np.zeros((128, NCC, 4, 65), np.float32)
        for cc in range(NCC):
            for i, h in enumerate(hs):
                wv[:, cc, i, 0:64] = Wv[h][cc * 128:cc * 128 + 128]

        wp = np.empty((128, 2, C), np.float32)
        for pr in range(2):
            ha, hb = hs[2 * pr], hs[2 * pr + 1]
            wp[0:64, pr] = Wp[64 * ha:64 * ha + 64]
            wp[64:128, pr] = Wp[64 * hb:64 * hb + 64]

        in_maps.append({"xT": xT, "wqk": wqk, "wv": wv, "wp": wp,
                        "ones": np.ones((128, NJ, 4, 1), np.float32)})
    return in_maps


def run(x, Wq, Wk, Wv, Wp, trace=False):
    nc = get_program()
    in_maps = prep_in_maps(x, Wq, Wk, Wv, Wp)
    res = bass_utils.run_bass_kernel_spmd(
        nc, in_maps, core_ids=list(range(8)), trace=trace)
    ys = [np.asarray(r["y"]) for r in res.results]
    out = np.stack([ys[2 * b] + ys[2 * b + 1] for b in range(B)])
    return out, res


def kernel(x, Wq, Wk, Wv, Wp):
    out, _ = run(x, Wq, Wk, Wv, Wp)
    return out


# revision 20
# speedup vs baseline: 1.0051x; 1.0051x over previous
"""Multi-head causal attention (B=4, T=2048, C=512, H=8, HS=64) on 8 TRN2 cores.

Sharding: 2D (batch x head-group). Core (b, g) = core 2*b+g handles batch b and
heads 4g..4g+3, producing a partial output y_bg = sum_h softmax(q_h k_h^T) v_h
@ Wp[64h:64h+64]. Host sums the two head-group partials per batch.

Per-core kernel layout (all matmuls fp32r, contraction dim on partitions):
  - x^T  [c, t] supplied pre-transposed from host (c-chunked) so projections
    contract over c without on-chip transposes.
  - q^T/k^T [d, t] per head-pair stacked on partitions (head a rows 0:64,
    head b rows 64:128) -> scores S^T[tk, tq] = k-block @ q^T via
    lhsT=k^T-block, rhs=q^T (both operands at the same base partition).
  - softmax without max-subtraction (scores ~ N(0,1): exp is safe);
    exp only over the causally-valid column range; per-chunk triangle zeroed
    with gpsimd affine_select; denominators come free as row 64 of the
    attention output via a ones-column appended to v (lhsT M=65).
  - out^T normalized by 1/sums (DVE reciprocal_approx_fast + stride-0-DMA
    partition broadcast), then y[tq, c] = (outn-pair).T @ Wp-pair with two
    heads stacked on the contraction dim (K=128).
"""

import numpy as np

import concourse.bass as bass
import concourse.mybir as mybir
import concourse.tile as tile
from concourse import bacc
from concourse import bass_utils

F32 = mybir.dt.float32
F32R = mybir.dt.float32r

B, T, C, H, HS = 4, 2048, 512, 8, 64
NCC = 4          # 128-row chunks of C
NJ = 16          # 128-row tk chunks of T
NS = 4           # 512-col tq slices of T
SCALE = HS ** -0.5


def r(ap):
    return ap


def build_mha_kernel(tc):
    nc = tc.nc
    xT_d = nc.dram_tensor("xT", [128, NCC, T], F32R, kind="ExternalInput").ap()
    wqk_d = nc.dram_tensor("wqk", [128, 2, 2, NCC, 128], F32R, kind="ExternalInput").ap()
    wv_d = nc.dram_tensor("wv", [128, NCC, 4, 65], F32R, kind="ExternalInput").ap()
    wp_d = nc.dram_tensor("wp", [64, 4, C], F32R, kind="ExternalInput").ap()
    ones_d = nc.dram_tensor("ones", [128, NJ, 4, 1], F32R, kind="ExternalInput").ap()
    y_d = nc.dram_tensor("y", [T, C], F32, kind="ExternalOutput").ap()

    with (
        tc.tile_pool(name="big", bufs=1) as bigp,
        tc.tile_pool(name="pslab", bufs=2) as pslabp,
        tc.tile_pool(name="outup", bufs=2) as outup,
        tc.tile_pool(name="recipp", bufs=2) as recipp,
        tc.tile_pool(name="ysb", bufs=3) as ysbp,
    ):
        xs = bigp.tile([128, NCC, T], F32R)
        for cc in range(NCC):
            nc.sync.dma_start(out=xs[:, cc, :], in_=xT_d[:, cc, :])
        wqk_s = bigp.tile([128, 2, 2, NCC, 128], F32R)
        nc.sync.dma_start(out=wqk_s, in_=wqk_d)
        wv_s = bigp.tile([128, NCC, 4, 65], F32R)
        nc.sync.dma_start(out=wv_s, in_=wv_d)
        wp_s = bigp.tile([64, 4, C], F32R)
        nc.sync.dma_start(out=wp_s, in_=wp_d)

        qT = bigp.tile([128, 2, T], F32R)      # [d-pair, pr, t]
        kT = bigp.tile([128, 2, T], F32R)
        v4e = bigp.tile([128, NJ, 4, 65], F32R)  # [t-in-chunk, j, head, d|1]
        outn = bigp.tile([64, 4, T], F32R)    # normalized out^T per head

        # ---- q/k projections: two heads packed in M ----
        with tc.tile_pool(name="pj", bufs=4, space="PSUM") as pjp:
            for pr in range(2):
                for qk in range(2):
                    tiles = []
                    for ts in range(NS):
                        qk_ps = pjp.tile([128, 512], F32, tag="qk", name=f"qk_{pr}_{qk}_{ts}")
                        tiles.append(qk_ps)
                    for cc in range(NCC):
                        for ts in range(NS):
                            nc.tensor.matmul(
                                tiles[ts],
                                r(wqk_s[:, pr, qk, cc, :]),
                                r(xs[:, cc, 512 * ts:512 * (ts + 1)]),
                                start=(cc == 0), stop=(cc == NCC - 1),
                            )
                    dst = qT if qk == 0 else kT
                    for ts in range(NS):
                        nc.vector.tensor_copy(
                            out=dst[:, pr, 512 * ts:512 * (ts + 1)], in_=tiles[ts])

            # ---- v projection: 4 heads packed in N (260 cols) ----
            for j in range(NJ):
                v_ps = pjp.tile([128, 4, 65], F32, tag="v", name=f"v_{j}")
                for cc in range(NCC):
                    nc.tensor.matmul(
                        v_ps,
                        r(xs[:, cc, 128 * j:128 * (j + 1)]),
                        r(wv_s[:, cc, :, :]),
                        start=(cc == 0), stop=(cc == NCC - 1),
                    )
                nc.vector.tensor_copy(out=v4e[:, j, :, :], in_=v_ps)
            # ones column for the av-sums row (after the copies that overwrite it)
            nc.sync.dma_start(out=v4e[:, :, :, 64:65], in_=ones_d)

        # ---- attention, head by head, tq in two 1024-col halves ----
        with (
            tc.tile_pool(name="otp", bufs=2, space="PSUM") as otp,
            tc.tile_pool(name="spp", bufs=2, space="PSUM") as spp,
        ):
            for hh in range(4):
                pr, lo = hh // 2, (hh % 2) * 64
                q_h = qT[lo:lo + 64, pr, :]
                k_h = kT[lo:lo + 64, pr, :]
                outu = outup.tile([65, T], F32, tag="outu", name=f"outu_{hh}")

                for half in range(2):
                    ot = otp.tile([65, 1024], F32, tag="ot", name=f"ot_{hh}_{half}")
                    jmax = 8 if half == 0 else NJ
                    svals = [2 * half, 2 * half + 1]  # global tq slices
                    p_tiles = [None] * jmax

                    def emit_scores(j):
                        smin = j // 4
                        stile = spp.tile([128, 1024], F32, tag="s",
                                         name=f"s_{hh}_{half}_{j}")
                        for s in svals:
                            if s < smin:
                                continue
                            nc.tensor.matmul(
                                stile[:, (s % 2) * 512:(s % 2) * 512 + 512],
                                r(k_h[:, 128 * j:128 * (j + 1)]),
                                r(q_h[:, 512 * s:512 * (s + 1)]),
                                start=True, stop=True,
                            )
                        pt = pslabp.tile([128, 1024], F32R, tag="p",
                                         name=f"p_{hh}_{half}_{j}")
                        p_tiles[j] = pt
                        st = max(0, 128 * j - 1024 * half)
                        nc.scalar.activation(
                            out=pt[:, st:1024], in_=stile[:, st:1024],
                            func=mybir.ActivationFunctionType.Exp,
                            scale=SCALE,
                        )
                        if smin in svals:
                            m = j % 4
                            lz = 512 * (smin - 2 * half)
                            if m > 0:
                                # zero cols left of the diagonal block
                                # (predicate always false -> fill)
                                nc.gpsimd.affine_select(
                                    out=pt[:, lz:lz + 128 * m],
                                    in_=xs[:, 0, 0:128 * m],
                                    compare_op=mybir.AluOpType.is_ge,
                                    fill=0.0, base=-1, channel_multiplier=0,
                                    pattern=[[0, 128 * m]],
                                )
                            # zero upper triangle of the diagonal block:
                            # keep (i, jj) iff jj >= i
                            nc.gpsimd.affine_select(
                                out=pt[:, st:st + 128],
                                in_=pt[:, st:st + 128],
                                compare_op=mybir.AluOpType.is_ge,
                                fill=0.0, base=0, channel_multiplier=-1,
                                pattern=[[1, 128]],
                            )

                    def emit_av(j):
                        pt = p_tiles[j]
                        for s in svals:
                            if s < j // 4:
                                continue
                            nc.tensor.matmul(
                                ot[:, (s % 2) * 512:(s % 2) * 512 + 512],
                                r(v4e[:, j, hh, :]),
                                r(pt[:, (s % 2) * 512:(s % 2) * 512 + 512]),
                                start=(j == 0), stop=(j == 4 * s + 3),
                            )

                    emit_scores(0)
                    for j in range(1, jmax):
                        emit_scores(j)
                        emit_av(j - 1)
                    emit_av(jmax - 1)

                    # stash raw out^T (+ sums row 64) for this half to SBUF
                    nc.vector.tensor_copy(
                        out=outu[:, 1024 * half:1024 * (half + 1)], in_=ot)

                # sums row -> partition 0 (recip/broadcast ignore AP partition
                # offsets on HW), reciprocal, broadcast to 64 partitions
                sums0 = recipp.tile([1, T], F32, tag="sums0", name=f"sums0_{hh}")
                nc.sync.dma_start(out=sums0, in_=outu[64:65, :])
                nc.vector.reciprocal_approx_fast(out=sums0, in_=sums0)
                recipb = recipp.tile([64, T], F32, tag="recipb", name=f"recipb_{hh}")
                nc.gpsimd.partition_broadcast(recipb, sums0)
                nc.vector.tensor_mul(outn[:, hh, :], outu[0:64, :], recipb)

        # ---- output projection: y[tq, c] = sum_pr outn_pr.T @ wp_pr ----
        with tc.tile_pool(name="yp", bufs=3, space="PSUM") as ypp:
            for ch in range(NJ):
                y_ps = ypp.tile([128, C], F32, tag="y", name=f"y_{ch}")
                for i in range(4):
                    nc.tensor.matmul(
                        y_ps,
                        r(outn[:, i, 128 * ch:128 * (ch + 1)]),
                        r(wp_s[:, i, :]),
                        start=(i == 0), stop=(i == 3),
                    )
                y_sb = ysbp.tile([128, C], F32, tag="ysb", name=f"ysb_{ch}")
                nc.vector.tensor_copy(out=y_sb, in_=y_ps)
                nc.sync.dma_start(out=y_d[128 * ch:128 * (ch + 1), :], in_=y_sb)


_PROGRAM = None


def get_program():
    global _PROGRAM
    if _PROGRAM is None:
        nc = bacc.Bacc("TRN2", target_bir_lowering=False, debug=False)
        with tile.TileContext(nc) as tc:
            build_mha_kernel(tc)
        nc.compile()
        _PROGRAM = nc
    return _PROGRAM


def prep_in_maps(x, Wq, Wk, Wv, Wp):
    x = np.asarray(x, np.float32)
    Wq = np.asarray(Wq, np.float32)
    Wk = np.asarray(Wk, np.float32)
    Wv = np.asarray(Wv, np.float32)
    Wp = np.asarray(Wp, np.float32)

    in_maps = []
    for core in range(8):
        b, g = core // 2, core % 2
        hs = [4 * g + i for i in range(4)]

        xT = np.ascontiguousarray(
            x[b].T.reshape(NCC, 128, T).transpose(1, 0, 2))  # [128, cc, t]

        wqk = np.empty((128, 2, 2, NCC, 128), np.float32)
        for pr in range(2):
            ha, hb = hs[2 * pr], hs[2 * pr + 1]
            for cc in range(NCC):
                sl = slice(cc * 128, cc * 128 + 128)
                wqk[:, pr, 0, cc, 0:64] = Wq[ha][sl]
                wqk[:, pr, 0, cc, 64:128] = Wq[hb][sl]
                wqk[:, pr, 1, cc, 0:64] = Wk[ha][sl]
                wqk[:, pr, 1, cc, 64:128] = Wk[hb][sl]

        wv = np.zeros((128, NCC, 4, 65), np.float32)
        for cc in range(NCC):
            for i, h in enumerate(hs):
                wv[:, cc, i, 0:64] = Wv[h][cc * 128:cc * 128 + 128]

        wp = np.empty((64, 4, C), np.float32)
        for i, h in enumerate(hs):
            wp[:, i, :] = Wp[64 * h:64 * h + 64]

        in_maps.append({"xT": xT, "wqk": wqk, "wv": wv, "wp": wp,
                        "ones": np.ones((128, NJ, 4, 1), np.float32)})
    return in_maps


def run(x, Wq, Wk, Wv, Wp, trace=False):
    nc = get_program()
    in_maps = prep_in_maps(x, Wq, Wk, Wv, Wp)
    res = bass_utils.run_bass_kernel_spmd(
        nc, in_maps, core_ids=list(range(8)), trace=trace)
    ys = [np.asarray(r["y"]) for r in res.results]
    out = np.stack([ys[2 * b] + ys[2 * b + 1] for b in range(B)])
    return out, res


def kernel(x, Wq, Wk, Wv, Wp):
    out, _ = run(x, Wq, Wk, Wv, Wp)
    return out


# revision 27
# speedup vs baseline: 1.0397x; 1.0344x over previous
"""Multi-head causal attention (B=4, T=2048, C=512, H=8, HS=64) on 8 TRN2 cores.

Sharding: 2D (batch x head-group). Core (b, g) = core 2*b+g handles batch b and
heads 4g..4g+3, producing a partial output y_bg = sum_h softmax(q_h k_h^T) v_h
@ Wp[64h:64h+64]. Host sums the two head-group partials per batch.

Per-core kernel layout (all matmuls fp32r, contraction dim on partitions):
  - x^T  [c, t] supplied pre-transposed from host (c-chunked) so projections
    contract over c without on-chip transposes.
  - q^T/k^T [d, t] per head-pair stacked on partitions (head a rows 0:64,
    head b rows 64:128) -> scores S^T[tk, tq] = k-block @ q^T via
    lhsT=k^T-block, rhs=q^T (both operands at the same base partition).
  - softmax without max-subtraction (scores ~ N(0,1): exp is safe);
    exp only over the causally-valid column range; per-chunk triangle zeroed
    with gpsimd affine_select; denominators come free as row 64 of the
    attention output via a ones-column appended to v (lhsT M=65).
  - out^T normalized by 1/sums (DVE reciprocal_approx_fast + stride-0-DMA
    partition broadcast), then y[tq, c] = (outn-pair).T @ Wp-pair with two
    heads stacked on the contraction dim (K=128).
"""

import numpy as np

import concourse.bass as bass
import concourse.mybir as mybir
import concourse.tile as tile
from concourse import bacc
from concourse import bass_utils

F32 = mybir.dt.float32
F32R = mybir.dt.float32r

B, T, C, H, HS = 4, 2048, 512, 8, 64
NCC = 4          # 128-row chunks of C
NJ = 16          # 128-row tk chunks of T
NS = 4           # 512-col tq slices of T
SCALE = HS ** -0.5


def r(ap):
    return ap


def build_mha_kernel(tc):
    nc = tc.nc
    xT_d = nc.dram_tensor("xT", [128, NCC, T], F32R, kind="ExternalInput").ap()
    wqk_d = nc.dram_tensor("wqk", [128, 2, 2, NCC, 128], F32R, kind="ExternalInput").ap()
    wv_d = nc.dram_tensor("wv", [128, NCC, 4, 65], F32R, kind="ExternalInput").ap()
    wp_d = nc.dram_tensor("wp", [64, 4, C], F32R, kind="ExternalInput").ap()
    ones_d = nc.dram_tensor("ones", [128, 4, 65], F32R, kind="ExternalInput").ap()
    y_d = nc.dram_tensor("y", [T, C], F32, kind="ExternalOutput").ap()

    with (
        tc.tile_pool(name="big", bufs=1) as bigp,
        tc.tile_pool(name="pslab", bufs=3) as pslabp,
        tc.tile_pool(name="outup", bufs=2) as outup,
        tc.tile_pool(name="recipp", bufs=2) as recipp,
        tc.tile_pool(name="ysb", bufs=3) as ysbp,
    ):
        xs = bigp.tile([128, NCC, T], F32R)
        for cc in range(NCC):
            nc.sync.dma_start(out=xs[:, cc, :], in_=xT_d[:, cc, :])
        wqk_s = bigp.tile([128, 2, 2, NCC, 128], F32R)
        nc.sync.dma_start(out=wqk_s, in_=wqk_d)
        wv_s = bigp.tile([128, NCC, 4, 65], F32R)
        nc.sync.dma_start(out=wv_s, in_=wv_d)
        wp_s = bigp.tile([64, 4, C], F32R)
        nc.sync.dma_start(out=wp_s, in_=wp_d)
        mask65 = bigp.tile([128, 4, 65], F32R)
        nc.sync.dma_start(out=mask65, in_=ones_d)

        qT = bigp.tile([128, 2, T], F32R)      # [d-pair, pr, t]
        kT = bigp.tile([128, 2, T], F32R)
        v4e = bigp.tile([128, NJ, 4, 65], F32R)  # [t-in-chunk, j, head, d|1]
        outn = bigp.tile([64, 4, T], F32R)    # normalized out^T per head

        # ---- q/k projections: two heads packed in M ----
        with tc.tile_pool(name="pj", bufs=4, space="PSUM") as pjp:
            for pr in range(2):
                for qk in range(2):
                    tiles = []
                    for ts in range(NS):
                        qk_ps = pjp.tile([128, 512], F32, tag="qk", name=f"qk_{pr}_{qk}_{ts}")
                        tiles.append(qk_ps)
                    for cc in range(NCC):
                        for ts in range(NS):
                            nc.tensor.matmul(
                                tiles[ts],
                                r(wqk_s[:, pr, qk, cc, :]),
                                r(xs[:, cc, 512 * ts:512 * (ts + 1)]),
                                start=(cc == 0), stop=(cc == NCC - 1),
                            )
                    dst = qT if qk == 0 else kT
                    for ts in range(NS):
                        nc.vector.tensor_copy(
                            out=dst[:, pr, 512 * ts:512 * (ts + 1)], in_=tiles[ts])

            # ---- v projection: 4 heads packed in N (260 cols) ----
            for j in range(NJ):
                v_ps = pjp.tile([128, 4, 65], F32, tag="v", name=f"v_{j}")
                for cc in range(NCC):
                    nc.tensor.matmul(
                        v_ps,
                        r(xs[:, cc, 128 * j:128 * (j + 1)]),
                        r(wv_s[:, cc, :, :]),
                        start=(cc == 0), stop=(cc == NCC - 1),
                    )
                # copy + inject the ones column (mask is 1.0 at col 64 only)
                nc.vector.tensor_add(v4e[:, j, :, :], v_ps, mask65)

        # ---- attention, head by head, tq in two 1024-col halves ----
        with (
            tc.tile_pool(name="otp", bufs=1, space="PSUM") as otp,
            tc.tile_pool(name="spp", bufs=3, space="PSUM") as spp,
        ):
            for hh in range(4):
                pr, lo = hh // 2, (hh % 2) * 64
                q_h = qT[lo:lo + 64, pr, :]
                k_h = kT[lo:lo + 64, pr, :]
                outu = outup.tile([65, T], F32, tag="outu", name=f"outu_{hh}")

                for half in range(2):
                    ot = otp.tile([65, 1024], F32, tag="ot", name=f"ot_{hh}_{half}")
                    jmax = 8 if half == 0 else NJ
                    svals = [2 * half, 2 * half + 1]  # global tq slices
                    p_tiles = [None] * jmax

                    def emit_scores(j):
                        smin = j // 4
                        stile = spp.tile([128, 1024], F32, tag="s",
                                         name=f"s_{hh}_{half}_{j}")
                        for s in svals:
                            if s < smin:
                                continue
                            nc.tensor.matmul(
                                stile[:, (s % 2) * 512:(s % 2) * 512 + 512],
                                r(k_h[:, 128 * j:128 * (j + 1)]),
                                r(q_h[:, 512 * s:512 * (s + 1)]),
                                start=True, stop=True,
                            )
                        pt = pslabp.tile([128, 1024], F32R, tag="p",
                                         name=f"p_{hh}_{half}_{j}")
                        p_tiles[j] = pt
                        st = max(0, 128 * j - 1024 * half)
                        nc.scalar.activation(
                            out=pt[:, st:1024], in_=stile[:, st:1024],
                            func=mybir.ActivationFunctionType.Exp,
                            scale=SCALE,
                        )
                        if smin in svals:
                            m = j % 4
                            lz = 512 * (smin - 2 * half)
                            if m > 0:
                                # zero cols left of the diagonal block
                                # (predicate always false -> fill)
                                nc.gpsimd.affine_select(
                                    out=pt[:, lz:lz + 128 * m],
                                    in_=xs[:, 0, 0:128 * m],
                                    compare_op=mybir.AluOpType.is_ge,
                                    fill=0.0, base=-1, channel_multiplier=0,
                                    pattern=[[0, 128 * m]],
                                )
                            # zero upper triangle of the diagonal block:
                            # keep (i, jj) iff jj >= i
                            nc.gpsimd.affine_select(
                                out=pt[:, st:st + 128],
                                in_=pt[:, st:st + 128],
                                compare_op=mybir.AluOpType.is_ge,
                                fill=0.0, base=0, channel_multiplier=-1,
                                pattern=[[1, 128]],
                            )

                    def emit_av(j):
                        pt = p_tiles[j]
                        for s in svals:
                            if s < j // 4:
                                continue
                            nc.tensor.matmul(
                                ot[:, (s % 2) * 512:(s % 2) * 512 + 512],
                                r(v4e[:, j, hh, :]),
                                r(pt[:, (s % 2) * 512:(s % 2) * 512 + 512]),
                                start=(j == 0), stop=(j == 4 * s + 3),
                            )

                    emit_scores(0)
                    emit_scores(1)
                    for j in range(2, jmax):
                        emit_scores(j)
                        emit_av(j - 2)
                    emit_av(jmax - 2)
                    emit_av(jmax - 1)

                    # stash raw out^T (+ sums row 64) for this half to SBUF
                    nc.vector.tensor_copy(
                        out=outu[:, 1024 * half:1024 * (half + 1)], in_=ot)

                # sums row -> partition 0 (recip/broadcast ignore AP partition
                # offsets on HW), reciprocal, broadcast to 64 partitions
                sums0 = recipp.tile([1, T], F32, tag="sums0", name=f"sums0_{hh}")
                nc.sync.dma_start(out=sums0, in_=outu[64:65, :])
                nc.vector.reciprocal_approx_fast(out=sums0, in_=sums0)
                recipb = recipp.tile([64, T], F32, tag="recipb", name=f"recipb_{hh}")
                nc.gpsimd.partition_broadcast(recipb, sums0)
                nc.vector.tensor_mul(outn[:, hh, :], outu[0:64, :], recipb)

        # ---- output projection: y[tq, c] = sum_pr outn_pr.T @ wp_pr ----
        with tc.tile_pool(name="yp", bufs=3, space="PSUM") as ypp:
            for ch in range(NJ):
                y_ps = ypp.tile([128, C], F32, tag="y", name=f"y_{ch}")
                for i in range(4):
                    nc.tensor.matmul(
                        y_ps,
                        r(outn[:, i, 128 * ch:128 * (ch + 1)]),
                        r(wp_s[:, i, :]),
                        start=(i == 0), stop=(i == 3),
                    )
                y_sb = ysbp.tile([128, C], F32, tag="ysb", name=f"ysb_{ch}")
                nc.vector.tensor_copy(out=y_sb, in_=y_ps)
                nc.sync.dma_start(out=y_d[128 * ch:128 * (ch + 1), :], in_=y_sb)


_PROGRAM = None


def get_program():
    global _PROGRAM
    if _PROGRAM is None:
        nc = bacc.Bacc("TRN2", target_bir_lowering=False, debug=False)
        with tile.TileContext(nc) as tc:
            build_mha_kernel(tc)
        nc.compile()
        _PROGRAM = nc
    return _PROGRAM


def prep_in_maps(x, Wq, Wk, Wv, Wp):
    x = np.asarray(x, np.float32)
    Wq = np.asarray(Wq, np.float32)
    Wk = np.asarray(Wk, np.float32)
    Wv = np.asarray(Wv, np.float32)
    Wp = np.asarray(Wp, np.float32)

    in_maps = []
    for core in range(8):
        b, g = core // 2, core % 2
        hs = [4 * g + i for i in range(4)]

        xT = np.ascontiguousarray(
            x[b].T.reshape(NCC, 128, T).transpose(1, 0, 2))  # [128, cc, t]

        wqk = np.empty((128, 2, 2, NCC, 128), np.float32)
        for pr in range(2):
            ha, hb = hs[2 * pr], hs[2 * pr + 1]
            for cc in range(NCC):
                sl = slice(cc * 128, cc * 128 + 128)
                wqk[:, pr, 0, cc, 0:64] = Wq[ha][sl]
                wqk[:, pr, 0, cc, 64:128] = Wq[hb][sl]
                wqk[:, pr, 1, cc, 0:64] = Wk[ha][sl]
                wqk[:, pr, 1, cc, 64:128] = Wk[hb][sl]

        wv = np.zeros((128, NCC, 4, 65), np.float32)
        for cc in range(NCC):
            for i, h in enumerate(hs):
                wv[:, cc, i, 0:64] = Wv[h][cc * 128:cc * 128 + 128]

        wp = np.empty((64, 4, C), np.float32)
        for i, h in enumerate(hs):
            wp[:, i, :] = Wp[64 * h:64 * h + 64]

        ones = np.zeros((128, 4, 65), np.float32)
        ones[:, :, 64] = 1.0
        in_maps.append({"xT": xT, "wqk": wqk, "wv": wv, "wp": wp, "ones": ones})
    return in_maps


def run(x, Wq, Wk, Wv, Wp, trace=False):
    nc = get_program()
    in_maps = prep_in_maps(x, Wq, Wk, Wv, Wp)
    res = bass_utils.run_bass_kernel_spmd(
        nc, in_maps, core_ids=list(range(8)), trace=trace)
    ys = [np.asarray(r["y"]) for r in res.results]
    out = np.stack([ys[2 * b] + ys[2 * b + 1] for b in range(B)])
    return out, res


def kernel(x, Wq, Wk, Wv, Wp):
    out, _ = run(x, Wq, Wk, Wv, Wp)
    return out


# revision 29
# speedup vs baseline: 1.0721x; 1.0312x over previous
"""Multi-head causal attention (B=4, T=2048, C=512, H=8, HS=64) on 8 TRN2 cores.

Sharding: 2D (batch x head-group). Core (b, g) = core 2*b+g handles batch b and
heads 4g..4g+3, producing a partial output y_bg = sum_h softmax(q_h k_h^T) v_h
@ Wp[64h:64h+64]. Host sums the two head-group partials per batch.

Per-core kernel layout (all matmuls fp32r, contraction dim on partitions):
  - x^T  [c, t] supplied pre-transposed from host (c-chunked) so projections
    contract over c without on-chip transposes.
  - q^T/k^T [d, t] per head-pair stacked on partitions (head a rows 0:64,
    head b rows 64:128) -> scores S^T[tk, tq] = k-block @ q^T via
    lhsT=k^T-block, rhs=q^T (both operands at the same base partition).
  - softmax without max-subtraction (scores ~ N(0,1): exp is safe);
    exp only over the causally-valid column range; per-chunk triangle zeroed
    with gpsimd affine_select; denominators come free as row 64 of the
    attention output via a ones-column appended to v (lhsT M=65).
  - out^T normalized by 1/sums (DVE reciprocal_approx_fast + stride-0-DMA
    partition broadcast), then y[tq, c] = (outn-pair).T @ Wp-pair with two
    heads stacked on the contraction dim (K=128).
"""

import numpy as np

import concourse.bass as bass
import concourse.mybir as mybir
import concourse.tile as tile
from concourse import bacc
from concourse import bass_utils

F32 = mybir.dt.float32
F32R = mybir.dt.float32r

B, T, C, H, HS = 4, 2048, 512, 8, 64
NCC = 4          # 128-row chunks of C
NJ = 16          # 128-row tk chunks of T
NS = 4           # 512-col tq slices of T
SCALE = HS ** -0.5


def r(ap):
    return ap


def build_mha_kernel(tc):
    nc = tc.nc
    xT_d = nc.dram_tensor("xT", [128, NCC, T], F32R, kind="ExternalInput").ap()
    wqk_d = nc.dram_tensor("wqk", [128, 2, 2, NCC, 128], F32R, kind="ExternalInput").ap()
    wv_d = nc.dram_tensor("wv", [128, NCC, 4, 65], F32R, kind="ExternalInput").ap()
    wp_d = nc.dram_tensor("wp", [64, 4, C], F32R, kind="ExternalInput").ap()
    ones_d = nc.dram_tensor("ones", [128, 4, 65], F32R, kind="ExternalInput").ap()
    y_d = nc.dram_tensor("y", [T, C], F32, kind="ExternalOutput").ap()

    with (
        tc.tile_pool(name="big", bufs=1) as bigp,
        tc.tile_pool(name="pslab", bufs=4) as pslabp,
        tc.tile_pool(name="outup", bufs=2) as outup,
        tc.tile_pool(name="recipp", bufs=2) as recipp,
        tc.tile_pool(name="ysb", bufs=3) as ysbp,
    ):
        xs = bigp.tile([128, NCC, T], F32R)
        for cc in range(NCC):
            nc.sync.dma_start(out=xs[:, cc, :], in_=xT_d[:, cc, :])
        wqk_s = bigp.tile([128, 2, 2, NCC, 128], F32R)
        nc.sync.dma_start(out=wqk_s, in_=wqk_d)
        wv_s = bigp.tile([128, NCC, 4, 65], F32R)
        nc.sync.dma_start(out=wv_s, in_=wv_d)
        wp_s = bigp.tile([64, 4, C], F32R)
        nc.sync.dma_start(out=wp_s, in_=wp_d)
        mask65 = bigp.tile([128, 4, 65], F32R)
        nc.sync.dma_start(out=mask65, in_=ones_d)

        qT = bigp.tile([128, 2, T], F32R)      # [d-pair, pr, t]
        kT = bigp.tile([128, 2, T], F32R)
        v4e = bigp.tile([128, NJ, 4, 65], F32R)  # [t-in-chunk, j, head, d|1]
        outn = bigp.tile([64, 4, T], F32R)    # normalized out^T per head

        # ---- q/k projections: two heads packed in M ----
        with tc.tile_pool(name="pj", bufs=4, space="PSUM") as pjp:
            for pr in range(2):
                for qk in range(2):
                    tiles = []
                    for ts in range(NS):
                        qk_ps = pjp.tile([128, 512], F32, tag="qk", name=f"qk_{pr}_{qk}_{ts}")
                        tiles.append(qk_ps)
                    for cc in range(NCC):
                        for ts in range(NS):
                            nc.tensor.matmul(
                                tiles[ts],
                                r(wqk_s[:, pr, qk, cc, :]),
                                r(xs[:, cc, 512 * ts:512 * (ts + 1)]),
                                start=(cc == 0), stop=(cc == NCC - 1),
                            )
                    dst = qT if qk == 0 else kT
                    for ts in range(NS):
                        nc.vector.tensor_copy(
                            out=dst[:, pr, 512 * ts:512 * (ts + 1)], in_=tiles[ts])

            # ---- v projection: 4 heads packed in N (260 cols) ----
            for j in range(NJ):
                v_ps = pjp.tile([128, 4, 65], F32, tag="v", name=f"v_{j}")
                for cc in range(NCC):
                    nc.tensor.matmul(
                        v_ps,
                        r(xs[:, cc, 128 * j:128 * (j + 1)]),
                        r(wv_s[:, cc, :, :]),
                        start=(cc == 0), stop=(cc == NCC - 1),
                    )
                # copy + inject the ones column (mask is 1.0 at col 64 only)
                nc.vector.tensor_add(v4e[:, j, :, :], v_ps, mask65)

        # ---- attention: two heads interleaved per pass to hide latency ----
        with (
            tc.tile_pool(name="otp", bufs=2, space="PSUM") as otp,
            tc.tile_pool(name="spp", bufs=2, space="PSUM") as spp,
        ):
            for pair in range(2):
                heads = [2 * pair, 2 * pair + 1]
                outus = {}
                for hh in heads:
                    outus[hh] = outup.tile([65, T], F32, tag="outu",
                                           name=f"outu_{hh}")

                for half in range(2):
                    jmax = 8 if half == 0 else NJ
                    svals = [2 * half, 2 * half + 1]  # global tq slices
                    ots = {}
                    for hh in heads:
                        ots[hh] = otp.tile([65, 1024], F32, tag="ot",
                                           name=f"ot_{hh}_{half}")
                    p_tiles = {hh: [None] * jmax for hh in heads}

                    def emit_scores(hh, j):
                        pr, lo = hh // 2, (hh % 2) * 64
                        q_h = qT[lo:lo + 64, pr, :]
                        k_h = kT[lo:lo + 64, pr, :]
                        smin = j // 4
                        stile = spp.tile([128, 1024], F32, tag="s",
                                         name=f"s_{hh}_{half}_{j}")
                        for s in svals:
                            if s < smin:
                                continue
                            nc.tensor.matmul(
                                stile[:, (s % 2) * 512:(s % 2) * 512 + 512],
                                r(k_h[:, 128 * j:128 * (j + 1)]),
                                r(q_h[:, 512 * s:512 * (s + 1)]),
                                start=True, stop=True,
                            )
                        pt = pslabp.tile([128, 1024], F32R, tag="p",
                                         name=f"p_{hh}_{half}_{j}")
                        p_tiles[hh][j] = pt
                        st = max(0, 128 * j - 1024 * half)
                        nc.scalar.activation(
                            out=pt[:, st:1024], in_=stile[:, st:1024],
                            func=mybir.ActivationFunctionType.Exp,
                            scale=SCALE,
                        )
                        if smin in svals:
                            m = j % 4
                            lz = 512 * (smin - 2 * half)
                            if m > 0:
                                # zero cols left of the diagonal block
                                # (predicate always false -> fill)
                                nc.gpsimd.affine_select(
                                    out=pt[:, lz:lz + 128 * m],
                                    in_=xs[:, 0, 0:128 * m],
                                    compare_op=mybir.AluOpType.is_ge,
                                    fill=0.0, base=-1, channel_multiplier=0,
                                    pattern=[[0, 128 * m]],
                                )
                            # zero upper triangle of the diagonal block:
                            # keep (i, jj) iff jj >= i
                            nc.gpsimd.affine_select(
                                out=pt[:, st:st + 128],
                                in_=pt[:, st:st + 128],
                                compare_op=mybir.AluOpType.is_ge,
                                fill=0.0, base=0, channel_multiplier=-1,
                                pattern=[[1, 128]],
                            )

                    def emit_av(hh, j):
                        pt = p_tiles[hh][j]
                        for s in svals:
                            if s < j // 4:
                                continue
                            nc.tensor.matmul(
                                ots[hh][:, (s % 2) * 512:(s % 2) * 512 + 512],
                                r(v4e[:, j, hh, :]),
                                r(pt[:, (s % 2) * 512:(s % 2) * 512 + 512]),
                                start=(j == 0), stop=(j == 4 * s + 3),
                            )

                    for hh in heads:
                        emit_scores(hh, 0)
                    for j in range(1, jmax):
                        for hh in heads:
                            emit_scores(hh, j)
                        for hh in heads:
                            emit_av(hh, j - 1)
                    for hh in heads:
                        emit_av(hh, jmax - 1)

                    # stash raw out^T (+ sums row 64) for this half to SBUF
                    for hh in heads:
                        nc.vector.tensor_copy(
                            out=outus[hh][:, 1024 * half:1024 * (half + 1)],
                            in_=ots[hh])

                for hh in heads:
                    # sums row -> partition 0 (recip/broadcast ignore AP
                    # partition offsets on HW), reciprocal, broadcast
                    outu = outus[hh]
                    sums0 = recipp.tile([1, T], F32, tag="sums0",
                                        name=f"sums0_{hh}")
                    nc.sync.dma_start(out=sums0, in_=outu[64:65, :])
                    nc.vector.reciprocal_approx_fast(out=sums0, in_=sums0)
                    recipb = recipp.tile([64, T], F32, tag="recipb",
                                         name=f"recipb_{hh}")
                    nc.gpsimd.partition_broadcast(recipb, sums0)
                    nc.vector.tensor_mul(outn[:, hh, :], outu[0:64, :], recipb)

        # ---- output projection: y[tq, c] = sum_pr outn_pr.T @ wp_pr ----
        with tc.tile_pool(name="yp", bufs=3, space="PSUM") as ypp:
            for ch in range(NJ):
                y_ps = ypp.tile([128, C], F32, tag="y", name=f"y_{ch}")
                for i in range(4):
                    nc.tensor.matmul(
                        y_ps,
                        r(outn[:, i, 128 * ch:128 * (ch + 1)]),
                        r(wp_s[:, i, :]),
                        start=(i == 0), stop=(i == 3),
                    )
                y_sb = ysbp.tile([128, C], F32, tag="ysb", name=f"ysb_{ch}")
                nc.vector.tensor_copy(out=y_sb, in_=y_ps)
                nc.sync.dma_start(out=y_d[128 * ch:128 * (ch + 1), :], in_=y_sb)


_PROGRAM = None


def get_program():
    global _PROGRAM
    if _PROGRAM is None:
        nc = bacc.Bacc("TRN2", target_bir_lowering=False, debug=False)
        with tile.TileContext(nc) as tc:
            build_mha_kernel(tc)
        nc.compile()
        _PROGRAM = nc
    return _PROGRAM


def prep_in_maps(x, Wq, Wk, Wv, Wp):
    x = np.asarray(x, np.float32)
    Wq = np.asarray(Wq, np.float32)
    Wk = np.asarray(Wk, np.float32)
    Wv = np.asarray(Wv, np.float32)
    Wp = np.asarray(Wp, np.float32)

    in_maps = []
    for core in range(8):
        b, g = core // 2, core % 2
        hs = [4 * g + i for i in range(4)]

        xT = np.ascontiguousarray(
            x[b].T.reshape(NCC, 128, T).transpose(1, 0, 2))  # [128, cc, t]

        wqk = np.empty((128, 2, 2, NCC, 128), np.float32)
        for pr in range(2):
            ha, hb = hs[2 * pr], hs[2 * pr + 1]
            for cc in range(NCC):
                sl = slice(cc * 128, cc * 128 + 128)
                wqk[:, pr, 0, cc, 0:64] = Wq[ha][sl]
                wqk[:, pr, 0, cc, 64:128] = Wq[hb][sl]
                wqk[:, pr, 1, cc, 0:64] = Wk[ha][sl]
                wqk[:, pr, 1, cc, 64:128] = Wk[hb][sl]

        wv = np.zeros((128, NCC, 4, 65), np.float32)
        for cc in range(NCC):
            for i, h in enumerate(hs):
                wv[:, cc, i, 0:64] = Wv[h][cc * 128:cc * 128 + 128]

        wp = np.empty((64, 4, C), np.float32)
        for i, h in enumerate(hs):
            wp[:, i, :] = Wp[64 * h:64 * h + 64]

        ones = np.zeros((128, 4, 65), np.float32)
        ones[:, :, 64] = 1.0
        in_maps.append({"xT": xT, "wqk": wqk, "wv": wv, "wp": wp, "ones": ones})
    return in_maps


def run(x, Wq, Wk, Wv, Wp, trace=False):
    nc = get_program()
    in_maps = prep_in_maps(x, Wq, Wk, Wv, Wp)
    res = bass_utils.run_bass_kernel_spmd(
        nc, in_maps, core_ids=list(range(8)), trace=trace)
    ys = [np.asarray(r["y"]) for r in res.results]
    out = np.stack([ys[2 * b] + ys[2 * b + 1] for b in range(B)])
    return out, res


def kernel(x, Wq, Wk, Wv, Wp):
    out, _ = run(x, Wq, Wk, Wv, Wp)
    return out
